# revision 5
# baseline (speedup 1.0000x reference)
"""Trainium2 Bass kernel for nn_FLB_Attention_Layer (gated fusion + additive
attention over 3 tokens + output projection, with residuals).

Strategy: pure data-parallel over batch B=4096 across 8 NeuronCores
(512 samples/core, weights replicated). Inside each core:

- Host pre-transposes tokens to feature-major f16 and weights to W.T column
  blocks [16 ot, 128 in-part, 16 k, 128 out]; gate/Q/K/V weights pre-scaled
  by 512 and quantized to fp8e4 on host.
- Gate + Q/K/V matmuls run in fp8e4 with perf_mode=DoubleRow (two 128-deep
  k-tiles per instruction) into a [P,3,512] PSUM trio (k outer, token
  inner); one wide ACT eviction per projection folds the 1/512 de-scale.
- Additive attention per head: T = tanh(q_i + k_j) via one broadcast DVE add
  + one wide ACT tanh per query i; scores via f16 matmul with lhsT = v_a[h]
  replicated across columns (broadcasts scores to every partition row);
  softmax with batched reciprocal_approx_fast; weighted v sum on DVE.
- Output projection in f16 (accuracy-critical path), residual added from
  re-streamed tokens, stored feature-major f32; host un-transposes.
"""

import numpy as np

P = 128
D = 2048
H = 16
DH = 128
KT = D // P  # 16 k-tiles
B = 4096
N_CORES = 8
B_C = B // N_CORES  # 512 per core
WS = 512.0  # fp8 weight pre-scale (power of 2)

_compiled = {}


def _build(b_c=B_C, d=D, h=H):
    import concourse.bass as bass
    import concourse.mybir as mybir
    import concourse.tile as tile
    from contextlib import ExitStack
    from concourse import bacc

    f32 = mybir.dt.float32
    f16 = mybir.dt.float16
    fp8 = mybir.dt.float8e4
    AF = mybir.ActivationFunctionType
    DR = mybir.MatmulPerfMode.DoubleRow

    kt = d // P
    nh = h

    nc = bacc.Bacc(None, target_bir_lowering=False, debug=False)

    # ---- params (all host-side pre-laid-out) ----
    tokf = nc.declare_dram_parameter("tokf", [3, P, kt, b_c], f16, isOutput=False)
    W8 = {
        name: nc.declare_dram_parameter(name, [kt, P, kt, P], fp8, isOutput=False)
        for name in ("WgL", "WgX", "Wq", "Wk", "Wv")
    }
    Wo16 = nc.declare_dram_parameter("Wo", [kt, P, kt, P], f16, isOutput=False)
    bgLT = nc.declare_dram_parameter("bgLT", [P, kt], f32, isOutput=False)
    bgXT = nc.declare_dram_parameter("bgXT", [P, kt], f32, isOutput=False)
    vaR = nc.declare_dram_parameter("vaR", [DH, nh, P], f16, isOutput=False)
    out = nc.declare_dram_parameter("out", [P, 3, kt, b_c], f32, isOutput=True)

    inv = 1.0 / WS

    with tile.TileContext(nc) as tc:
        with ExitStack() as ctx:
            const = ctx.enter_context(tc.tile_pool(name="const", bufs=1))
            ptok = ctx.enter_context(tc.tile_pool(name="ptok", bufs=1))
            pstr = ctx.enter_context(tc.tile_pool(name="pstr", bufs=4))
            pw8 = ctx.enter_context(tc.tile_pool(name="pw8", bufs=3))
            pwo = ctx.enter_context(tc.tile_pool(name="pwo", bufs=2))
            pqkv = ctx.enter_context(tc.tile_pool(name="pqkv", bufs=3))
            ptt = ctx.enter_context(tc.tile_pool(name="ptt", bufs=2))
            psm = ctx.enter_context(tc.tile_pool(name="psm", bufs=2))
            pden = ctx.enter_context(tc.tile_pool(name="pden", bufs=1))
            pout = ctx.enter_context(tc.tile_pool(name="pout", bufs=2))
            ps_tri = ctx.enter_context(tc.tile_pool(name="ps_tri", bufs=2, space="PSUM"))
            ps_sc = ctx.enter_context(tc.tile_pool(name="ps_sc", bufs=2, space="PSUM"))

            bgl_t = const.tile([P, kt], f32)
            bgx_t = const.tile([P, kt], f32)
            nc.sync.dma_start(bgl_t[:], bgLT[:])
            nc.sync.dma_start(bgx_t[:], bgXT[:])
            va_rep = const.tile([DH, nh, P], f16)
            nc.sync.dma_start(va_rep[:], vaR[:])

            # fdbk kept resident f16 (for quantize + residual); x/lat streamed
            fdbkF = ptok.tile([P, kt, b_c], f16)
            nc.sync.dma_start(fdbkF[:], tokf[2])
            tok8 = ptok.tile([P, 3, kt, b_c], fp8)
            for q in range(4):
                nc.vector.tensor_copy(
                    tok8[:, 2, 4 * q : 4 * q + 4, :], fdbkF[:, 4 * q : 4 * q + 4, :]
                )

            # attention output collected feature-major (k-tile == head)
            attT = ptok.tile([P, kt, 3, b_c], f16)

            def load_w8(name, ot):
                wT = pw8.tile([P, kt, P], fp8, tag="w8")
                nc.sync.dma_start(wT[:], W8[name][ot])
                return wT

            def mm_fp8_tri(ps, wT, tokens):
                """k-outer / token-inner DoubleRow accumulation into psum trio."""
                for k in range(0, kt, 2):
                    for t in tokens:
                        nc.tensor.matmul(
                            ps[:, t, :],
                            wT[:, k : k + 2, :],
                            tok8[:, t, k : k + 2, :],
                            start=(k == 0),
                            stop=(k == kt - 2),
                            perf_mode=DR,
                        )

            # ---- phase 1: gated fusion ----
            # G_L = sigmoid(fdbk @ WgL.T + bgL); lat' = lat * G_L
            # G_X = sigmoid(lat' @ WgX.T + bgX); x' = x * G_X
            for stage, (wname, bg_t, src_tok, dst_tok) in enumerate(
                [("WgL", bgl_t, 2, 1), ("WgX", bgx_t, 1, 0)]
            ):
                for ot in range(kt):
                    wT = load_w8(wname, ot)
                    dstF = pstr.tile([P, b_c], f16, tag="tokstr")
                    nc.sync.dma_start(dstF[:], tokf[dst_tok, :, ot, :])
                    pg = ps_tri.tile([P, 3, b_c], f32, tag="tri")
                    mm_fp8_tri(pg, wT, (src_tok,))
                    gate = psm.tile([P, b_c], f16, tag="gate")
                    nc.scalar.activation(
                        gate[:], pg[:, src_tok, :], AF.Sigmoid,
                        bias=bg_t[:, ot : ot + 1], scale=inv,
                    )
                    nc.vector.tensor_mul(tok8[:, dst_tok, ot, :], dstF[:], gate[:])

            # ---- phase 2: per-head QKV + additive attention ----
            # Software-pipelined: head h-1's score matmuls + exps are emitted
            # between head h's projection groups so PE always has DoubleRow
            # work queued while ACT drains, and the sc-psum ping-pong hides.
            def emit_scores(st, lo, hi):
                hh, Tt, _, E9 = st
                for ij in range(lo, hi):
                    i, j = ij // 3, ij % 3
                    sc = ps_sc.tile([P, b_c], f32, tag="sc")
                    nc.tensor.matmul(
                        sc[:],
                        va_rep[:, hh, :],
                        Tt[:, 3 * i + j, :],
                        start=True,
                        stop=True,
                    )
                    nc.scalar.activation(E9[:, j, i, :], sc[:], AF.Exp)

            def emit_softmax(st):
                hh, _, vh, E9 = st
                den3 = pden.tile([P, 3, b_c], f32, tag="den3")
                rden3 = pden.tile([P, 3, b_c], f32, tag="rden3")
                nc.vector.tensor_add(den3[:], E9[:, 0, :, :], E9[:, 1, :, :])
                nc.vector.tensor_add(den3[:], den3[:], E9[:, 2, :, :])
                nc.vector.reciprocal_approx_fast(rden3[:], den3[:])
                for i in range(3):
                    acc = psm.tile([P, b_c], f16, tag="acc")
                    tmp = psm.tile([P, b_c], f16, tag="tmp")
                    nc.vector.tensor_mul(acc[:], vh[:, 0, :], E9[:, 0, i, :])
                    nc.vector.tensor_mul(tmp[:], vh[:, 1, :], E9[:, 1, i, :])
                    nc.vector.tensor_add(acc[:], acc[:], tmp[:])
                    nc.vector.tensor_mul(tmp[:], vh[:, 2, :], E9[:, 2, i, :])
                    nc.vector.tensor_add(acc[:], acc[:], tmp[:])
                    nc.vector.tensor_mul(attT[:, hh, i, :], acc[:], rden3[:, i, :])

            pending = None
            for hh in range(nh):
                qkv_sb = []
                for pi, wname in enumerate(("Wq", "Wk", "Wv")):
                    wT = load_w8(wname, hh)
                    pp = ps_tri.tile([P, 3, b_c], f32, tag="tri")
                    mm_fp8_tri(pp, wT, (0, 1, 2))
                    dst = pqkv.tile([P, 3, b_c], f16, tag=f"qkv{pi}")
                    nc.scalar.activation(dst[:], pp[:], AF.Copy, scale=inv)
                    qkv_sb.append(dst)
                    if pending is not None:
                        emit_scores(pending, 3 * pi, 3 * pi + 3)
                if pending is not None:
                    emit_softmax(pending)
                qh, kh, vh = qkv_sb

                # T = tanh(q_i + k_j), f16 [p, ij, b]
                Tt = ptt.tile([P, 9, b_c], f16, tag="Tt")
                for i in range(3):
                    pre3 = psm.tile([P, 3, b_c], f16, tag="pre3")
                    nc.vector.tensor_add(
                        pre3[:], qh[:, i : i + 1, :].broadcast_to([P, 3, b_c]), kh[:]
                    )
                    nc.scalar.activation(Tt[:, 3 * i : 3 * i + 3, :], pre3[:], AF.Tanh)

                E9 = ptt.tile([P, 3, 3, b_c], f16, tag="E9")  # [p, j, i, b]
                pending = (hh, Tt, vh, E9)

            emit_scores(pending, 0, 9)
            emit_softmax(pending)

            # ---- phase 3: output projection (f16) + residual ----
            for ot in range(kt):
                wT = pwo.tile([P, kt, P], f16, tag="wo")
                nc.sync.dma_start(wT[:], Wo16[ot])
                po = ps_tri.tile([P, 3, b_c], f32, tag="tri")
                for k in range(kt):
                    for t in range(3):
                        nc.tensor.matmul(
                            po[:, t, :],
                            wT[:, k, :],
                            attT[:, k, t, :],
                            start=(k == 0),
                            stop=(k == kt - 1),
                        )
                for t in range(3):
                    if t == 2:
                        resF = fdbkF[:, ot, :]
                    else:
                        rt = pstr.tile([P, b_c], f16, tag="tokstr")
                        nc.sync.dma_start(rt[:], tokf[t, :, ot, :])
                        resF = rt[:]
                    oT = pout.tile([P, b_c], f32, tag="oT")
                    nc.vector.tensor_add(oT[:], po[:, t, :], resF)
                    nc.sync.dma_start(out[:, t, ot, :], oT[:])

    nc.compile()
    return nc


def _get_nc():
    key = "full"
    if key not in _compiled:
        _compiled[key] = _build()
    return _compiled[key]


def kernel(
    x_token,
    lat_token,
    fdbk_token,
    W_gate_L,
    b_gate_L,
    W_gate_X,
    b_gate_X,
    W_q,
    W_k,
    W_v,
    W_o,
    v_a,
):
    import ml_dtypes
    from concourse.bass_utils import run_bass_kernel_spmd

    nc = _get_nc()

    f32 = np.float32
    f16 = np.float16
    fp8 = ml_dtypes.float8_e4m3

    def wblocks(W, dtype, scale=1.0):
        # [ot, p, k, o] = W[ot*128+o, k*128+p] * scale
        a = (np.asarray(W, f32) * scale).reshape(KT, P, KT, P).transpose(0, 3, 2, 1)
        return np.ascontiguousarray(a).astype(dtype)

    w8 = {
        "WgL": wblocks(W_gate_L, fp8, WS),
        "WgX": wblocks(W_gate_X, fp8, WS),
        "Wq": wblocks(W_q, fp8, WS),
        "Wk": wblocks(W_k, fp8, WS),
        "Wv": wblocks(W_v, fp8, WS),
    }
    wo = wblocks(W_o, f16)
    bglT = np.ascontiguousarray(np.asarray(b_gate_L, f32).reshape(KT, P).T)
    bgxT = np.ascontiguousarray(np.asarray(b_gate_X, f32).reshape(KT, P).T)
    va = np.asarray(v_a, f32).reshape(H, DH)  # [h, d]
    vaR = np.ascontiguousarray(
        np.broadcast_to(va.T[:, :, None], (DH, H, P))
    ).astype(f16)

    # tokens feature-major f16: [3, P, KT, B] then per-core batch slice
    toks = np.stack(
        [
            np.asarray(t, f32).reshape(B, KT, P).transpose(2, 1, 0)
            for t in (x_token, lat_token, fdbk_token)
        ],
        axis=0,
    ).astype(f16)  # [3, P, KT, B]

    in_maps = []
    for c in range(N_CORES):
        s = slice(c * B_C, (c + 1) * B_C)
        m = {
            "tokf": np.ascontiguousarray(toks[:, :, :, s]),
            "Wo": wo,
            "bgLT": bglT,
            "bgXT": bgxT,
            "vaR": vaR,
        }
        m.update(w8)
        in_maps.append(m)

    res = run_bass_kernel_spmd(nc, in_maps, list(range(N_CORES))).results

    # out [P, 3, KT, B_C] f32 -> [B_C, 3, D]
    full = np.concatenate(
        [res[c]["out"].transpose(3, 1, 2, 0).reshape(B_C, 3, D) for c in range(N_CORES)],
        axis=0,
    )
    return tuple(np.ascontiguousarray(full[:, t : t + 1, :]) for t in range(3))


# revision 27
# speedup vs baseline: 1.1529x; 1.1529x over previous
"""Trainium2 Bass kernel for nn_FLB_Attention_Layer (gated fusion + additive
attention over 3 tokens + output projection, with residuals).

Strategy: pure data-parallel over batch B=4096 across 8 NeuronCores
(512 samples/core, weights replicated). Inside each core:

- Host pre-transposes tokens to feature-major f16 and weights to W.T column
  blocks [16 ot, 128 in-part, 16 k, 128 out]; gate/Q/K/V weights pre-scaled
  by 512 and quantized to fp8e4 on host.
- Gate + Q/K/V matmuls run in fp8e4 with perf_mode=DoubleRow (two 128-deep
  k-tiles per instruction) into a [P,3,512] PSUM trio (k outer, token
  inner); one wide ACT eviction per projection folds the 1/512 de-scale.
- Additive attention per head: T = tanh(q_i + k_j) via one broadcast DVE add
  + one wide ACT tanh per query i; scores via f16 matmul with lhsT = v_a[h]
  replicated across columns (broadcasts scores to every partition row);
  softmax with batched reciprocal_approx_fast; weighted v sum on DVE.
- Output projection in f16 (accuracy-critical path), residual added from
  re-streamed tokens, stored feature-major f32; host un-transposes.
"""

import numpy as np

P = 128
D = 2048
H = 16
DH = 128
KT = D // P  # 16 k-tiles
B = 4096
N_CORES = 8
B_C = B // N_CORES  # 512 per core
WS = 512.0  # fp8 weight pre-scale (power of 2)

_compiled = {}


def _build(b_c=B_C, d=D, h=H):
    import concourse.bass as bass
    import concourse.mybir as mybir
    import concourse.tile as tile
    from contextlib import ExitStack
    from concourse import bacc

    f32 = mybir.dt.float32
    f16 = mybir.dt.float16
    fp8 = mybir.dt.float8e4
    AF = mybir.ActivationFunctionType
    DR = mybir.MatmulPerfMode.DoubleRow

    kt = d // P
    nh = h

    nc = bacc.Bacc(None, target_bir_lowering=False, debug=False)

    # ---- params (all host-side pre-laid-out) ----
    tokf = nc.declare_dram_parameter("tokf", [3, P, kt, b_c], f16, isOutput=False)
    W8 = {
        name: nc.declare_dram_parameter(name, [kt, P, kt, P], fp8, isOutput=False)
        for name in ("WgL", "WgX", "Wq", "Wk", "Wv")
    }
    Wo8 = nc.declare_dram_parameter("Wo", [kt, P, kt, P], fp8, isOutput=False)
    tokR = nc.declare_dram_parameter("tokR", [P, kt, 3, b_c], f16, isOutput=False)
    bgLT = nc.declare_dram_parameter("bgLT", [P, kt], f32, isOutput=False)
    bgXT = nc.declare_dram_parameter("bgXT", [P, kt], f32, isOutput=False)
    vaR = nc.declare_dram_parameter("vaR", [DH, 2, nh, P], fp8, isOutput=False)
    out = nc.declare_dram_parameter("out", [P, 3, kt, b_c], f32, isOutput=True)

    inv = 1.0 / WS

    with tile.TileContext(nc) as tc:
        with ExitStack() as ctx:
            const = ctx.enter_context(tc.tile_pool(name="const", bufs=1))
            ptok = ctx.enter_context(tc.tile_pool(name="ptok", bufs=1))
            pstr = ctx.enter_context(tc.tile_pool(name="pstr", bufs=8))
            prt3 = ctx.enter_context(tc.tile_pool(name="prt3", bufs=4))
            pw8 = ctx.enter_context(tc.tile_pool(name="pw8", bufs=3))
            pwo = ctx.enter_context(tc.tile_pool(name="pwo", bufs=3))
            pqkv = ctx.enter_context(tc.tile_pool(name="pqkv", bufs=3))
            ptt = ctx.enter_context(tc.tile_pool(name="ptt", bufs=3))
            psm = ctx.enter_context(tc.tile_pool(name="psm", bufs=2))
            pden = ctx.enter_context(tc.tile_pool(name="pden", bufs=1))
            pout = ctx.enter_context(tc.tile_pool(name="pout", bufs=2))
            ps_mm = ctx.enter_context(tc.tile_pool(name="ps_mm", bufs=6, space="PSUM"))
            ps_sc = ctx.enter_context(tc.tile_pool(name="ps_sc", bufs=2, space="PSUM"))

            bgl_t = const.tile([P, kt], f32)
            bgx_t = const.tile([P, kt], f32)
            nc.sync.dma_start(bgl_t[:], bgLT[:])
            nc.sync.dma_start(bgx_t[:], bgXT[:])
            # [d, pair, h, col]: pair 0 = v_a[h] replicated (x512, fp8),
            # pair 1 = zeros -> lets the score matmul run in DoubleRow mode
            # (uniform PE perf mode with the QKV groups it interleaves with)
            va_rep = const.tile([DH, 2, nh, P], fp8)
            nc.sync.dma_start(va_rep[:], vaR[:])

            # fdbk kept resident f16 (for quantize + residual); x/lat streamed
            fdbkF = ptok.tile([P, kt, b_c], f16)
            nc.sync.dma_start(fdbkF[:], tokf[2])
            tok8 = [
                ptok.tile([P, kt, b_c], fp8, name=f"tok8_{t}") for t in range(3)
            ]
            for q in range(4):
                nc.vector.tensor_copy(
                    tok8[2][:, 4 * q : 4 * q + 4, :], fdbkF[:, 4 * q : 4 * q + 4, :]
                )

            # attention output collected feature-major (k-tile == head)
            attT = ptok.tile([P, kt, 3, b_c], fp8)

            def load_w8(name, ot):
                wT = pw8.tile([P, kt, P], fp8, tag="w8")
                nc.sync.dma_start(wT[:], W8[name][ot])
                return wT

            # deferred attention work, drained one item at a time between
            # matmul-group chunks so PE never waits on the ACT/DVE chains
            from collections import deque

            work = deque()

            def drain_one():
                if work:
                    work.popleft()()

            def mm_fp8(pps, wT, tokens, drain_every=2):
                """k-outer / token-inner DoubleRow accumulation, one psum
                bank per token (fine-grained slot release)."""
                for ki, k in enumerate(range(0, kt, 2)):
                    for ti, t in enumerate(tokens):
                        nc.tensor.matmul(
                            pps[ti][:],
                            wT[:, k : k + 2, :],
                            tok8[t][:, k : k + 2, :],
                            start=(k == 0),
                            stop=(k == kt - 2),
                            perf_mode=DR,
                        )
                    if ki % drain_every == drain_every - 1:
                        drain_one()

            # ---- phase 1: gated fusion ----
            # G_L = sigmoid(fdbk @ WgL.T + bgL); lat' = lat * G_L
            # G_X = sigmoid(lat' @ WgX.T + bgX); x' = x * G_X
            for stage, (wname, bg_t, src_tok, dst_tok) in enumerate(
                [("WgL", bgl_t, 2, 1), ("WgX", bgx_t, 1, 0)]
            ):
                for ot in range(kt):
                    wT = load_w8(wname, ot)
                    dstF = pstr.tile([P, b_c], f16, tag="tokstr")
                    nc.sync.dma_start(dstF[:], tokf[dst_tok, :, ot, :])
                    pg = ps_mm.tile([P, b_c], f32, tag="mm")
                    mm_fp8([pg], wT, (src_tok,))
                    gate = psm.tile([P, b_c], f16, tag="gate")
                    nc.scalar.activation(
                        gate[:], pg[:], AF.Sigmoid,
                        bias=bg_t[:, ot : ot + 1], scale=inv,
                    )
                    nc.vector.tensor_mul(tok8[dst_tok][:, ot, :], dstF[:], gate[:])

            # ---- phase 2: per-head QKV + additive attention ----
            # Software-pipelined: head h-1's score matmuls/exps/softmax are
            # queued as work items drained between matmul chunks of head h,
            # so each exp's latency hides behind queued DoubleRow work.
            def push_scores(st):
                hh, Tt, vh, E9 = st

                def score(ij):
                    i, j = ij // 3, ij % 3
                    sc = ps_sc.tile([P, b_c], f32, tag="sc")
                    nc.tensor.matmul(
                        sc[:],
                        va_rep[:, :, hh, :],
                        Tt[:, ij : ij + 2, :],
                        start=True,
                        stop=True,
                        perf_mode=DR,
                    )
                    nc.scalar.activation(E9[:, j, i, :], sc[:], AF.Exp, scale=inv)

                def softmax():
                    den3 = pden.tile([P, 3, b_c], f32, tag="den3")
                    rden3 = pden.tile([P, 3, b_c], f32, tag="rden3")
                    nc.gpsimd.tensor_add(den3[:], E9[:, 0, :, :], E9[:, 1, :, :])
                    nc.gpsimd.tensor_add(den3[:], den3[:], E9[:, 2, :, :])
                    nc.vector.reciprocal_approx_fast(rden3[:], den3[:])
                    for i in range(3):
                        acc = psm.tile([P, b_c], f16, tag="acc")
                        tmp = psm.tile([P, b_c], f16, tag="tmp")
                        nc.vector.tensor_mul(acc[:], vh[:, 0, :], E9[:, 0, i, :])
                        nc.vector.tensor_mul(tmp[:], vh[:, 1, :], E9[:, 1, i, :])
                        nc.vector.tensor_add(acc[:], acc[:], tmp[:])
                        nc.vector.tensor_mul(tmp[:], vh[:, 2, :], E9[:, 2, i, :])
                        nc.vector.tensor_add(acc[:], acc[:], tmp[:])
                        nc.vector.tensor_mul(attT[:, hh, i, :], acc[:], rden3[:, i, :])

                for ij in range(9):
                    work.append(lambda ij=ij: score(ij))
                work.append(softmax)

            def emit_proj(wname, hh, pi, drain_every=2):
                wT = load_w8(wname, hh)
                pps = [ps_mm.tile([P, b_c], f32, tag="mm", name=f"pp{t}") for t in range(3)]
                mm_fp8(pps, wT, (0, 1, 2), drain_every=drain_every)
                dst = pqkv.tile([P, 3, b_c], f16, tag=f"qkv{pi}")
                for t in range(3):
                    nc.scalar.activation(dst[:, t, :], pps[t][:], AF.Copy, scale=inv)
                return dst

            for hh in range(nh):
                qh = emit_proj("Wq", hh, 0)
                kh = emit_proj("Wk", hh, 1)

                # T = tanh(q_i + k_j), fp8 [p, ij, b] (+ zeroed pad slot 9 so
                # the DoubleRow score matmul's dead half never reads NaNs);
                # emitted before the V group so next iteration's drained
                # score matmuls never wait on this head's tanh.
                Tt = ptt.tile([P, 10, b_c], fp8, tag="Tt")
                nc.any.memset(Tt[:, 9, :], 0.0)
                for i in range(3):
                    pre3 = psm.tile([P, 3, b_c], f16, tag="pre3")
                    nc.vector.tensor_add(
                        pre3[:], qh[:, i : i + 1, :].broadcast_to([P, 3, b_c]), kh[:]
                    )
                    nc.scalar.activation(Tt[:, 3 * i : 3 * i + 3, :], pre3[:], AF.Tanh)

                if hh == nh - 1:
                    # queue the last head's own scores before its V group so
                    # they drain during it instead of stalling before Wo
                    E9 = ptt.tile([P, 3, 3, b_c], f16, tag="E9")  # [p, j, i, b]
                    vh = pqkv.tile([P, 3, b_c], f16, tag="qkv2")
                    push_scores((hh, Tt, vh, E9))
                    wT = load_w8("Wv", hh)
                    pps = [ps_mm.tile([P, b_c], f32, tag="mm", name=f"pv{t}") for t in range(3)]
                    mm_fp8(pps, wT, (0, 1, 2), drain_every=1)
                    for t in range(3):
                        nc.scalar.activation(vh[:, t, :], pps[t][:], AF.Copy, scale=inv)
                else:
                    vh = emit_proj("Wv", hh, 2)
                    E9 = ptt.tile([P, 3, 3, b_c], f16, tag="E9")  # [p, j, i, b]
                    push_scores((hh, Tt, vh, E9))

            # drain the remaining deferred attention items before Wo
            while work:
                drain_one()

            # ---- phase 3: output projection (fp8 DoubleRow) + residual ----
            # psum holds 512*out; residual tokens come pre-scaled by 512 and
            # the host divides the stored output by 512 (exact, power of 2).
            for ot in range(kt):
                wT = pwo.tile([P, kt, P], fp8, tag="wo")
                nc.sync.dma_start(wT[:], Wo8[ot])
                pos = [ps_mm.tile([P, b_c], f32, tag="mm", name=f"po{t}") for t in range(3)]
                for k in range(0, kt, 2):
                    for t in range(3):
                        nc.tensor.matmul(
                            pos[t][:],
                            wT[:, k : k + 2, :],
                            attT[:, k : k + 2, t, :],
                            start=(k == 0),
                            stop=(k == kt - 2),
                            perf_mode=DR,
                        )
                rt3 = prt3.tile([P, 3, b_c], f16, tag="rt3")
                nc.sync.dma_start(rt3[:], tokR[:, ot])
                for t in range(3):
                    oT = pout.tile([P, b_c], f32, tag="oT")
                    nc.vector.tensor_add(oT[:], pos[t][:], rt3[:, t, :])
                    nc.sync.dma_start(out[:, t, ot, :], oT[:])

    nc.compile()
    return nc


def _get_nc():
    key = "full"
    if key not in _compiled:
        _compiled[key] = _build()
    return _compiled[key]


def kernel(
    x_token,
    lat_token,
    fdbk_token,
    W_gate_L,
    b_gate_L,
    W_gate_X,
    b_gate_X,
    W_q,
    W_k,
    W_v,
    W_o,
    v_a,
):
    import ml_dtypes
    from concourse.bass_utils import run_bass_kernel_spmd

    nc = _get_nc()

    f32 = np.float32
    f16 = np.float16
    fp8 = ml_dtypes.float8_e4m3

    def wblocks(W, dtype, scale=1.0):
        # [ot, p, k, o] = W[ot*128+o, k*128+p] * scale
        a = (np.asarray(W, f32) * scale).reshape(KT, P, KT, P).transpose(0, 3, 2, 1)
        return np.ascontiguousarray(a).astype(dtype)

    w8 = {
        "WgL": wblocks(W_gate_L, fp8, WS),
        "WgX": wblocks(W_gate_X, fp8, WS),
        "Wq": wblocks(W_q, fp8, WS),
        "Wk": wblocks(W_k, fp8, WS),
        "Wv": wblocks(W_v, fp8, WS),
    }
    wo = wblocks(W_o, fp8, WS)
    bglT = np.ascontiguousarray(np.asarray(b_gate_L, f32).reshape(KT, P).T)
    bgxT = np.ascontiguousarray(np.asarray(b_gate_X, f32).reshape(KT, P).T)
    va = np.asarray(v_a, f32).reshape(H, DH)  # [h, d]
    vaR = np.zeros((DH, 2, H, P), f32)
    vaR[:, 0, :, :] = np.broadcast_to(va.T[:, :, None], (DH, H, P)) * WS
    vaR = vaR.astype(fp8)

    # tokens feature-major f16: [3, P, KT, B] then per-core batch slice
    toks = np.stack(
        [
            np.asarray(t, f32).reshape(B, KT, P).transpose(2, 1, 0)
            for t in (x_token, lat_token, fdbk_token)
        ],
        axis=0,
    ).astype(f16)  # [3, P, KT, B]

    toksR = np.ascontiguousarray(
        (toks.astype(f32) * WS).transpose(1, 2, 0, 3)
    ).astype(f16)  # pre-scaled residuals, [P, KT, 3, B]
    in_maps = []
    for c in range(N_CORES):
        s = slice(c * B_C, (c + 1) * B_C)
        m = {
            "tokf": np.ascontiguousarray(toks[:, :, :, s]),
            "tokR": np.ascontiguousarray(toksR[:, :, :, s]),
            "Wo": wo,
            "bgLT": bglT,
            "bgXT": bgxT,
            "vaR": vaR,
        }
        m.update(w8)
        in_maps.append(m)

    res = run_bass_kernel_spmd(nc, in_maps, list(range(N_CORES))).results

    # out [P, 3, KT, B_C] f32 -> [B_C, 3, D]
    full = np.concatenate(
        [res[c]["out"].transpose(3, 1, 2, 0).reshape(B_C, 3, D) for c in range(N_CORES)],
        axis=0,
    ) * np.float32(1.0 / WS)
    return tuple(np.ascontiguousarray(full[:, t : t + 1, :]) for t in range(3))


# revision 31
# speedup vs baseline: 1.3358x; 1.1586x over previous
"""Trainium2 Bass kernel for nn_FLB_Attention_Layer (gated fusion + additive
attention over 3 tokens + output projection, with residuals).

Strategy: pure data-parallel over batch B=4096 across 8 NeuronCores
(512 samples/core, weights replicated). Inside each core:

- Host pre-transposes tokens to feature-major f16 and weights to W.T column
  blocks [16 ot, 128 in-part, 16 k, 128 out]; gate/Q/K/V weights pre-scaled
  by 512 and quantized to fp8e4 on host.
- Gate + Q/K/V matmuls run in fp8e4 with perf_mode=DoubleRow (two 128-deep
  k-tiles per instruction) into a [P,3,512] PSUM trio (k outer, token
  inner); one wide ACT eviction per projection folds the 1/512 de-scale.
- Additive attention per head: T = tanh(q_i + k_j) via one broadcast DVE add
  + one wide ACT tanh per query i; scores via f16 matmul with lhsT = v_a[h]
  replicated across columns (broadcasts scores to every partition row);
  softmax with batched reciprocal_approx_fast; weighted v sum on DVE.
- Output projection in f16 (accuracy-critical path), residual added from
  re-streamed tokens, stored feature-major f32; host un-transposes.
"""

import numpy as np

P = 128
D = 2048
H = 16
DH = 128
KT = D // P  # 16 k-tiles
B = 4096
N_CORES = 8
B_C = B // N_CORES  # 512 per core
WS = 512.0  # fp8 weight pre-scale (power of 2)

_compiled = {}


def _build(b_c=B_C, d=D, h=H):
    import concourse.bass as bass
    import concourse.mybir as mybir
    import concourse.tile as tile
    from contextlib import ExitStack
    from concourse import bacc

    f32 = mybir.dt.float32
    f16 = mybir.dt.float16
    fp8 = mybir.dt.float8e4
    AF = mybir.ActivationFunctionType
    DR = mybir.MatmulPerfMode.DoubleRow

    kt = d // P
    nh = h

    nc = bacc.Bacc(None, target_bir_lowering=False, debug=False)

    # ---- params (all host-side pre-laid-out) ----
    tokf = nc.declare_dram_parameter("tokf", [3, P, kt, b_c], f16, isOutput=False)
    W8 = {
        name: nc.declare_dram_parameter(name, [kt, P, kt, P], fp8, isOutput=False)
        for name in ("WgL", "WgX", "Wq", "Wk", "Wv")
    }
    Wo8 = nc.declare_dram_parameter("Wo", [kt, P, kt, P], fp8, isOutput=False)
    tokR = nc.declare_dram_parameter("tokR", [P, kt, 3, b_c], f16, isOutput=False)
    bgLT = nc.declare_dram_parameter("bgLT", [P, kt], f32, isOutput=False)
    bgXT = nc.declare_dram_parameter("bgXT", [P, kt], f32, isOutput=False)
    vaR = nc.declare_dram_parameter("vaR", [DH, 2, nh, P], fp8, isOutput=False)
    out = nc.declare_dram_parameter("out", [P, 3, kt, b_c], f32, isOutput=True)

    inv = 1.0 / WS

    with tile.TileContext(nc) as tc:
        with ExitStack() as ctx:
            const = ctx.enter_context(tc.tile_pool(name="const", bufs=1))
            ptok = ctx.enter_context(tc.tile_pool(name="ptok", bufs=1))
            pstr = ctx.enter_context(tc.tile_pool(name="pstr", bufs=6))
            prt3 = ctx.enter_context(tc.tile_pool(name="prt3", bufs=4))
            pw8 = ctx.enter_context(tc.tile_pool(name="pw8", bufs=3))
            pwo = ctx.enter_context(tc.tile_pool(name="pwo", bufs=3))
            pqkv = ctx.enter_context(tc.tile_pool(name="pqkv", bufs=3))
            ptt = ctx.enter_context(tc.tile_pool(name="ptt", bufs=3))
            psm = ctx.enter_context(tc.tile_pool(name="psm", bufs=3))
            pden = ctx.enter_context(tc.tile_pool(name="pden", bufs=1))
            pout = ctx.enter_context(tc.tile_pool(name="pout", bufs=4))
            ps_mm = ctx.enter_context(tc.tile_pool(name="ps_mm", bufs=6, space="PSUM"))
            ps_sc = ctx.enter_context(tc.tile_pool(name="ps_sc", bufs=2, space="PSUM"))

            bgl_t = const.tile([P, kt], f32)
            bgx_t = const.tile([P, kt], f32)
            nc.sync.dma_start(bgl_t[:], bgLT[:])
            nc.sync.dma_start(bgx_t[:], bgXT[:])
            # [d, pair, h, col]: pair 0 = v_a[h] replicated (x512, fp8),
            # pair 1 = zeros -> lets the score matmul run in DoubleRow mode
            # (uniform PE perf mode with the QKV groups it interleaves with)
            va_rep = const.tile([DH, 2, nh, P], fp8)
            nc.sync.dma_start(va_rep[:], vaR[:])

            # fdbk kept resident f16 (for quantize + residual); x/lat streamed
            fdbkF = ptok.tile([P, kt, b_c], f16)
            nc.sync.dma_start(fdbkF[:], tokf[2])
            tok8 = [
                ptok.tile([P, kt, b_c], fp8, name=f"tok8_{t}") for t in range(3)
            ]
            for q in range(4):
                nc.vector.tensor_copy(
                    tok8[2][:, 4 * q : 4 * q + 4, :], fdbkF[:, 4 * q : 4 * q + 4, :]
                )

            # attention output collected feature-major (k-tile == head)
            attT = ptok.tile([P, kt, 3, b_c], fp8)

            def load_w8(name, ot):
                wT = pw8.tile([P, kt, P], fp8, tag="w8")
                nc.sync.dma_start(wT[:], W8[name][ot])
                return wT

            # deferred attention work, drained one item at a time between
            # matmul-group chunks so PE never waits on the ACT/DVE chains
            from collections import deque

            work = deque()

            def drain_one():
                if work:
                    work.popleft()()

            def mm_fp8(pps, wT, tokens, drain_every=2):
                """k-outer / token-inner DoubleRow accumulation, one psum
                bank per token (fine-grained slot release)."""
                for ki, k in enumerate(range(0, kt, 2)):
                    for ti, t in enumerate(tokens):
                        nc.tensor.matmul(
                            pps[ti][:],
                            wT[:, k : k + 2, :],
                            tok8[t][:, k : k + 2, :],
                            start=(k == 0),
                            stop=(k == kt - 2),
                            perf_mode=DR,
                        )
                    if ki % drain_every == drain_every - 1:
                        drain_one()

            # ---- phase 1: gated fusion ----
            # G_L = sigmoid(fdbk @ WgL.T + bgL); lat' = lat * G_L
            # G_X = sigmoid(lat' @ WgX.T + bgX); x' = x * G_X
            for stage, (wname, bg_t, src_tok, dst_tok) in enumerate(
                [("WgL", bgl_t, 2, 1), ("WgX", bgx_t, 1, 0)]
            ):
                for ot in range(kt):
                    wT = load_w8(wname, ot)
                    dstF = pstr.tile([P, b_c], f16, tag="tokstr")
                    nc.sync.dma_start(dstF[:], tokf[dst_tok, :, ot, :])
                    pg = ps_mm.tile([P, b_c], f32, tag="mm")
                    mm_fp8([pg], wT, (src_tok,))
                    gate = psm.tile([P, b_c], f16, tag="gate")
                    nc.scalar.activation(
                        gate[:], pg[:], AF.Sigmoid,
                        bias=bg_t[:, ot : ot + 1], scale=inv,
                    )
                    nc.vector.tensor_mul(tok8[dst_tok][:, ot, :], dstF[:], gate[:])

            # ---- phase 2: per-head QKV + additive attention ----
            # Software-pipelined: head h-1's score matmuls/exps/softmax are
            # queued as work items drained between matmul chunks of head h,
            # so each exp's latency hides behind queued DoubleRow work.
            def push_scores(st):
                hh, Tt, vh, E9 = st

                def score(ij):
                    i, j = ij // 3, ij % 3
                    sc = ps_sc.tile([P, b_c], f32, tag="sc")
                    nc.tensor.matmul(
                        sc[:],
                        va_rep[:, :, hh, :],
                        Tt[:, ij : ij + 2, :],
                        start=True,
                        stop=True,
                        perf_mode=DR,
                    )
                    nc.scalar.activation(E9[:, j, i, :], sc[:], AF.Exp, scale=inv)

                def softmax():
                    den3 = pden.tile([P, 3, b_c], f32, tag="den3")
                    rden3 = pden.tile([P, 3, b_c], f32, tag="rden3")
                    nc.vector.tensor_add(den3[:], E9[:, 0, :, :], E9[:, 1, :, :])
                    nc.vector.tensor_add(den3[:], den3[:], E9[:, 2, :, :])
                    nc.vector.reciprocal_approx_fast(rden3[:], den3[:])
                    for i in range(3):
                        acc = psm.tile([P, b_c], f16, tag="acc")
                        tmp = psm.tile([P, b_c], f16, tag="tmp")
                        nc.vector.tensor_mul(acc[:], vh[:, 0, :], E9[:, 0, i, :])
                        nc.vector.tensor_mul(tmp[:], vh[:, 1, :], E9[:, 1, i, :])
                        nc.vector.tensor_add(acc[:], acc[:], tmp[:])
                        nc.vector.tensor_mul(tmp[:], vh[:, 2, :], E9[:, 2, i, :])
                        nc.vector.tensor_add(acc[:], acc[:], tmp[:])
                        nc.vector.tensor_mul(attT[:, hh, i, :], acc[:], rden3[:, i, :])

                for ij in range(9):
                    work.append(lambda ij=ij: score(ij))
                work.append(softmax)

            def emit_proj(wname, hh, pi, drain_every=2):
                wT = load_w8(wname, hh)
                pps = [ps_mm.tile([P, b_c], f32, tag="mm", name=f"pp{t}") for t in range(3)]
                mm_fp8(pps, wT, (0, 1, 2), drain_every=drain_every)
                dst = pqkv.tile([P, 3, b_c], f16, tag=f"qkv{pi}")
                for t in range(3):
                    nc.scalar.activation(dst[:, t, :], pps[t][:], AF.Copy, scale=inv)
                return dst

            for hh in range(nh):
                qh = emit_proj("Wq", hh, 0)
                kh = emit_proj("Wk", hh, 1)

                # T = tanh(q_i + k_j), fp8 [p, ij, b] (+ zeroed pad slot 9 so
                # the DoubleRow score matmul's dead half never reads NaNs);
                # emitted before the V group so next iteration's drained
                # score matmuls never wait on this head's tanh.
                Tt = ptt.tile([P, 10, b_c], fp8, tag="Tt")
                nc.any.memset(Tt[:, 9, :], 0.0)
                for i in range(3):
                    pre3 = psm.tile([P, 3, b_c], f16, tag="pre3")
                    nc.vector.tensor_add(
                        pre3[:], qh[:, i : i + 1, :].broadcast_to([P, 3, b_c]), kh[:]
                    )
                    nc.scalar.activation(Tt[:, 3 * i : 3 * i + 3, :], pre3[:], AF.Tanh)

                if hh == nh - 1:
                    # queue the last head's own scores before its V group so
                    # they drain during it instead of stalling before Wo
                    E9 = ptt.tile([P, 3, 3, b_c], f16, tag="E9")  # [p, j, i, b]
                    vh = pqkv.tile([P, 3, b_c], f16, tag="qkv2")
                    push_scores((hh, Tt, vh, E9))
                    wT = load_w8("Wv", hh)
                    pps = [ps_mm.tile([P, b_c], f32, tag="mm", name=f"pv{t}") for t in range(3)]
                    mm_fp8(pps, wT, (0, 1, 2), drain_every=1)
                    for t in range(3):
                        nc.scalar.activation(vh[:, t, :], pps[t][:], AF.Copy, scale=inv)
                else:
                    vh = emit_proj("Wv", hh, 2)
                    E9 = ptt.tile([P, 3, 3, b_c], f16, tag="E9")  # [p, j, i, b]
                    push_scores((hh, Tt, vh, E9))

            # drain the remaining deferred attention items before Wo
            while work:
                drain_one()

            # ---- phase 3: output projection (fp8 DoubleRow) + residual ----
            # psum holds 512*out; residual tokens come pre-scaled by 512 and
            # the host divides the stored output by 512 (exact, power of 2).
            for ot in range(kt):
                wT = pwo.tile([P, kt, P], fp8, tag="wo")
                nc.sync.dma_start(wT[:], Wo8[ot])
                pos = [ps_mm.tile([P, b_c], f32, tag="mm", name=f"po{t}") for t in range(3)]
                for k in range(0, kt, 2):
                    for t in range(3):
                        nc.tensor.matmul(
                            pos[t][:],
                            wT[:, k : k + 2, :],
                            attT[:, k : k + 2, t, :],
                            start=(k == 0),
                            stop=(k == kt - 2),
                            perf_mode=DR,
                        )
                rt3 = prt3.tile([P, 3, b_c], f16, tag="rt3")
                nc.sync.dma_start(rt3[:], tokR[:, ot])
                for t in range(3):
                    oT = pout.tile([P, b_c], f32, tag="oT")
                    nc.vector.tensor_add(oT[:], pos[t][:], rt3[:, t, :])
                    nc.sync.dma_start(out[:, t, ot, :], oT[:])

    nc.compile()
    return nc


def _get_nc():
    key = "full"
    if key not in _compiled:
        _compiled[key] = _build()
    return _compiled[key]


def kernel(
    x_token,
    lat_token,
    fdbk_token,
    W_gate_L,
    b_gate_L,
    W_gate_X,
    b_gate_X,
    W_q,
    W_k,
    W_v,
    W_o,
    v_a,
):
    import ml_dtypes
    from concourse.bass_utils import run_bass_kernel_spmd

    nc = _get_nc()

    f32 = np.float32
    f16 = np.float16
    fp8 = ml_dtypes.float8_e4m3

    def wblocks(W, dtype, scale=1.0):
        # [ot, p, k, o] = W[ot*128+o, k*128+p] * scale
        a = (np.asarray(W, f32) * scale).reshape(KT, P, KT, P).transpose(0, 3, 2, 1)
        return np.ascontiguousarray(a).astype(dtype)

    w8 = {
        "WgL": wblocks(W_gate_L, fp8, WS),
        "WgX": wblocks(W_gate_X, fp8, WS),
        "Wq": wblocks(W_q, fp8, WS),
        "Wk": wblocks(W_k, fp8, WS),
        "Wv": wblocks(W_v, fp8, WS),
    }
    wo = wblocks(W_o, fp8, WS)
    bglT = np.ascontiguousarray(np.asarray(b_gate_L, f32).reshape(KT, P).T)
    bgxT = np.ascontiguousarray(np.asarray(b_gate_X, f32).reshape(KT, P).T)
    va = np.asarray(v_a, f32).reshape(H, DH)  # [h, d]
    vaR = np.zeros((DH, 2, H, P), f32)
    vaR[:, 0, :, :] = np.broadcast_to(va.T[:, :, None], (DH, H, P)) * WS
    vaR = vaR.astype(fp8)

    # tokens feature-major f16: [3, P, KT, B] then per-core batch slice
    toks = np.stack(
        [
            np.asarray(t, f32).reshape(B, KT, P).transpose(2, 1, 0)
            for t in (x_token, lat_token, fdbk_token)
        ],
        axis=0,
    ).astype(f16)  # [3, P, KT, B]

    toksR = np.ascontiguousarray(
        (toks.astype(f32) * WS).transpose(1, 2, 0, 3)
    ).astype(f16)  # pre-scaled residuals, [P, KT, 3, B]
    in_maps = []
    for c in range(N_CORES):
        s = slice(c * B_C, (c + 1) * B_C)
        m = {
            "tokf": np.ascontiguousarray(toks[:, :, :, s]),
            "tokR": np.ascontiguousarray(toksR[:, :, :, s]),
            "Wo": wo,
            "bgLT": bglT,
            "bgXT": bgxT,
            "vaR": vaR,
        }
        m.update(w8)
        in_maps.append(m)

    res = run_bass_kernel_spmd(nc, in_maps, list(range(N_CORES))).results

    # out [P, 3, KT, B_C] f32 -> [B_C, 3, D]
    full = np.concatenate(
        [res[c]["out"].transpose(3, 1, 2, 0).reshape(B_C, 3, D) for c in range(N_CORES)],
        axis=0,
    ) * np.float32(1.0 / WS)
    return tuple(np.ascontiguousarray(full[:, t : t + 1, :]) for t in range(3))


# revision 32
# speedup vs baseline: 1.3383x; 1.0019x over previous
"""Trainium2 Bass kernel for nn_FLB_Attention_Layer (gated fusion + additive
attention over 3 tokens + output projection, with residuals).

Strategy: pure data-parallel over batch B=4096 across 8 NeuronCores
(512 samples/core, weights replicated). Inside each core:

- Host pre-transposes tokens to feature-major f16 and weights to W.T column
  blocks [16 ot, 128 in-part, 16 k, 128 out]; gate/Q/K/V weights pre-scaled
  by 512 and quantized to fp8e4 on host.
- Gate + Q/K/V matmuls run in fp8e4 with perf_mode=DoubleRow (two 128-deep
  k-tiles per instruction) into a [P,3,512] PSUM trio (k outer, token
  inner); one wide ACT eviction per projection folds the 1/512 de-scale.
- Additive attention per head: T = tanh(q_i + k_j) via one broadcast DVE add
  + one wide ACT tanh per query i; scores via f16 matmul with lhsT = v_a[h]
  replicated across columns (broadcasts scores to every partition row);
  softmax with batched reciprocal_approx_fast; weighted v sum on DVE.
- Output projection in f16 (accuracy-critical path), residual added from
  re-streamed tokens, stored feature-major f32; host un-transposes.
"""

import numpy as np

P = 128
D = 2048
H = 16
DH = 128
KT = D // P  # 16 k-tiles
B = 4096
N_CORES = 8
B_C = B // N_CORES  # 512 per core
WS = 512.0  # fp8 weight pre-scale (power of 2)

_compiled = {}


def _build(b_c=B_C, d=D, h=H):
    import concourse.bass as bass
    import concourse.mybir as mybir
    import concourse.tile as tile
    from contextlib import ExitStack
    from concourse import bacc

    f32 = mybir.dt.float32
    f16 = mybir.dt.float16
    fp8 = mybir.dt.float8e4
    AF = mybir.ActivationFunctionType
    DR = mybir.MatmulPerfMode.DoubleRow

    kt = d // P
    nh = h

    nc = bacc.Bacc(None, target_bir_lowering=False, debug=False)

    # ---- params (all host-side pre-laid-out) ----
    tokf = nc.declare_dram_parameter("tokf", [3, P, kt, b_c], f16, isOutput=False)
    W8 = {
        name: nc.declare_dram_parameter(name, [kt, P, kt, P], fp8, isOutput=False)
        for name in ("WgL", "WgX", "Wq", "Wk", "Wv")
    }
    Wo8 = nc.declare_dram_parameter("Wo", [kt, P, kt, P], fp8, isOutput=False)
    tokR = nc.declare_dram_parameter("tokR", [P, kt, 3, b_c], f16, isOutput=False)
    bgLT = nc.declare_dram_parameter("bgLT", [P, kt], f32, isOutput=False)
    bgXT = nc.declare_dram_parameter("bgXT", [P, kt], f32, isOutput=False)
    vaR = nc.declare_dram_parameter("vaR", [DH, 2, nh, P], fp8, isOutput=False)
    out = nc.declare_dram_parameter("out", [P, 3, kt, b_c], f32, isOutput=True)

    inv = 1.0 / WS

    with tile.TileContext(nc) as tc:
        with ExitStack() as ctx:
            const = ctx.enter_context(tc.tile_pool(name="const", bufs=1))
            ptok = ctx.enter_context(tc.tile_pool(name="ptok", bufs=1))
            pstr = ctx.enter_context(tc.tile_pool(name="pstr", bufs=6))
            prt3 = ctx.enter_context(tc.tile_pool(name="prt3", bufs=4))
            pw8 = ctx.enter_context(tc.tile_pool(name="pw8", bufs=3))
            pwo = ctx.enter_context(tc.tile_pool(name="pwo", bufs=3))
            pqkv = ctx.enter_context(tc.tile_pool(name="pqkv", bufs=3))
            ptt = ctx.enter_context(tc.tile_pool(name="ptt", bufs=3))
            psm = ctx.enter_context(tc.tile_pool(name="psm", bufs=3))
            pden = ctx.enter_context(tc.tile_pool(name="pden", bufs=1))
            pout = ctx.enter_context(tc.tile_pool(name="pout", bufs=4))
            ps_mm = ctx.enter_context(tc.tile_pool(name="ps_mm", bufs=6, space="PSUM"))
            ps_sc = ctx.enter_context(tc.tile_pool(name="ps_sc", bufs=2, space="PSUM"))

            bgl_t = const.tile([P, kt], f32)
            bgx_t = const.tile([P, kt], f32)
            nc.sync.dma_start(bgl_t[:], bgLT[:])
            nc.sync.dma_start(bgx_t[:], bgXT[:])
            # [d, pair, h, col]: pair 0 = v_a[h] replicated (x512, fp8),
            # pair 1 = zeros -> lets the score matmul run in DoubleRow mode
            # (uniform PE perf mode with the QKV groups it interleaves with)
            va_rep = const.tile([DH, 2, nh, P], fp8)
            nc.sync.dma_start(va_rep[:], vaR[:])

            # fdbk kept resident f16 (for quantize + residual); x/lat streamed
            fdbkF = ptok.tile([P, kt, b_c], f16)
            nc.sync.dma_start(fdbkF[:], tokf[2])
            tok8 = [
                ptok.tile([P, kt, b_c], fp8, name=f"tok8_{t}") for t in range(3)
            ]
            for q in range(4):
                nc.vector.tensor_copy(
                    tok8[2][:, 4 * q : 4 * q + 4, :], fdbkF[:, 4 * q : 4 * q + 4, :]
                )

            # attention output collected feature-major (k-tile == head)
            attT = ptok.tile([P, kt, 3, b_c], fp8)

            def load_w8(name, ot):
                wT = pw8.tile([P, kt, P], fp8, tag="w8")
                nc.sync.dma_start(wT[:], W8[name][ot])
                return wT

            # deferred attention work, drained one item at a time between
            # matmul-group chunks so PE never waits on the ACT/DVE chains
            from collections import deque

            work = deque()
            dve_work = deque()

            def drain_one():
                if work:
                    work.popleft()()

            def mm_fp8(pps, wT, tokens, drain_every=2):
                """k-outer / token-inner DoubleRow accumulation, one psum
                bank per token (fine-grained slot release)."""
                for ki, k in enumerate(range(0, kt, 2)):
                    for ti, t in enumerate(tokens):
                        nc.tensor.matmul(
                            pps[ti][:],
                            wT[:, k : k + 2, :],
                            tok8[t][:, k : k + 2, :],
                            start=(k == 0),
                            stop=(k == kt - 2),
                            perf_mode=DR,
                        )
                    if ki % drain_every == drain_every - 1:
                        drain_one()

            # ---- phase 1: gated fusion ----
            # G_L = sigmoid(fdbk @ WgL.T + bgL); lat' = lat * G_L
            # G_X = sigmoid(lat' @ WgX.T + bgX); x' = x * G_X
            for stage, (wname, bg_t, src_tok, dst_tok) in enumerate(
                [("WgL", bgl_t, 2, 1), ("WgX", bgx_t, 1, 0)]
            ):
                for ot in range(kt):
                    wT = load_w8(wname, ot)
                    dstF = pstr.tile([P, b_c], f16, tag="tokstr")
                    nc.sync.dma_start(dstF[:], tokf[dst_tok, :, ot, :])
                    pg = ps_mm.tile([P, b_c], f32, tag="mm")
                    mm_fp8([pg], wT, (src_tok,))
                    gate = psm.tile([P, b_c], f16, tag="gate")
                    nc.scalar.activation(
                        gate[:], pg[:], AF.Sigmoid,
                        bias=bg_t[:, ot : ot + 1], scale=inv,
                    )
                    nc.vector.tensor_mul(tok8[dst_tok][:, ot, :], dstF[:], gate[:])

            # ---- phase 2: per-head QKV + additive attention ----
            # Software-pipelined: head h-1's score matmuls/exps/softmax are
            # queued as work items drained between matmul chunks of head h,
            # so each exp's latency hides behind queued DoubleRow work.
            def push_scores(st, last=False):
                hh, Tt, vh, E9 = st

                def score(ij):
                    i, j = ij // 3, ij % 3
                    sc = ps_sc.tile([P, b_c], f32, tag="sc")
                    nc.tensor.matmul(
                        sc[:],
                        va_rep[:, :, hh, :],
                        Tt[:, ij : ij + 2, :],
                        start=True,
                        stop=True,
                        perf_mode=DR,
                    )
                    nc.scalar.activation(E9[:, j, i, :], sc[:], AF.Exp, scale=inv)

                def softmax():
                    den3 = pden.tile([P, 3, b_c], f32, tag="den3")
                    rden3 = pden.tile([P, 3, b_c], f32, tag="rden3")
                    nc.vector.tensor_add(den3[:], E9[:, 0, :, :], E9[:, 1, :, :])
                    nc.vector.tensor_add(den3[:], den3[:], E9[:, 2, :, :])
                    nc.vector.reciprocal_approx_fast(rden3[:], den3[:])
                    for i in range(3):
                        acc = psm.tile([P, b_c], f16, tag="acc")
                        tmp = psm.tile([P, b_c], f16, tag="tmp")
                        nc.vector.tensor_mul(acc[:], vh[:, 0, :], E9[:, 0, i, :])
                        nc.vector.tensor_mul(tmp[:], vh[:, 1, :], E9[:, 1, i, :])
                        nc.vector.tensor_add(acc[:], acc[:], tmp[:])
                        nc.vector.tensor_mul(tmp[:], vh[:, 2, :], E9[:, 2, i, :])
                        nc.vector.tensor_add(acc[:], acc[:], tmp[:])
                        nc.vector.tensor_mul(attT[:, hh, i, :], acc[:], rden3[:, i, :])

                for ij in range(9):
                    work.append(lambda ij=ij: score(ij))
                if not last:
                    work.append(softmax)
                else:
                    # split into DVE-only pieces drained inside the first Wo
                    # group (no PE instructions -> no weight-path thrash)
                    holder = {}

                    def sm_den():
                        den3 = pden.tile([P, 3, b_c], f32, tag="den3")
                        rden3 = pden.tile([P, 3, b_c], f32, tag="rden3")
                        nc.vector.tensor_add(den3[:], E9[:, 0, :, :], E9[:, 1, :, :])
                        nc.vector.tensor_add(den3[:], den3[:], E9[:, 2, :, :])
                        nc.vector.reciprocal_approx_fast(rden3[:], den3[:])
                        holder["r"] = rden3

                    def sm_i(i):
                        rden3 = holder["r"]
                        acc = psm.tile([P, b_c], f16, tag="acc")
                        tmp = psm.tile([P, b_c], f16, tag="tmp")
                        nc.vector.tensor_mul(acc[:], vh[:, 0, :], E9[:, 0, i, :])
                        nc.vector.tensor_mul(tmp[:], vh[:, 1, :], E9[:, 1, i, :])
                        nc.vector.tensor_add(acc[:], acc[:], tmp[:])
                        nc.vector.tensor_mul(tmp[:], vh[:, 2, :], E9[:, 2, i, :])
                        nc.vector.tensor_add(acc[:], acc[:], tmp[:])
                        nc.vector.tensor_mul(attT[:, hh, i, :], acc[:], rden3[:, i, :])

                    dve_work.append(sm_den)
                    for i in range(3):
                        dve_work.append(lambda i=i: sm_i(i))

            def emit_proj(wname, hh, pi, drain_every=2):
                wT = load_w8(wname, hh)
                pps = [ps_mm.tile([P, b_c], f32, tag="mm", name=f"pp{t}") for t in range(3)]
                mm_fp8(pps, wT, (0, 1, 2), drain_every=drain_every)
                dst = pqkv.tile([P, 3, b_c], f16, tag=f"qkv{pi}")
                for t in range(3):
                    nc.scalar.activation(dst[:, t, :], pps[t][:], AF.Copy, scale=inv)
                return dst

            for hh in range(nh):
                qh = emit_proj("Wq", hh, 0)
                kh = emit_proj("Wk", hh, 1)

                # T = tanh(q_i + k_j), fp8 [p, ij, b] (+ zeroed pad slot 9 so
                # the DoubleRow score matmul's dead half never reads NaNs);
                # emitted before the V group so next iteration's drained
                # score matmuls never wait on this head's tanh.
                Tt = ptt.tile([P, 10, b_c], fp8, tag="Tt")
                nc.any.memset(Tt[:, 9, :], 0.0)
                for i in range(3):
                    pre3 = psm.tile([P, 3, b_c], f16, tag="pre3")
                    nc.vector.tensor_add(
                        pre3[:], qh[:, i : i + 1, :].broadcast_to([P, 3, b_c]), kh[:]
                    )
                    nc.scalar.activation(Tt[:, 3 * i : 3 * i + 3, :], pre3[:], AF.Tanh)

                if hh == nh - 1:
                    # queue the last head's own scores before its V group so
                    # they drain during it instead of stalling before Wo
                    E9 = ptt.tile([P, 3, 3, b_c], f16, tag="E9")  # [p, j, i, b]
                    vh = pqkv.tile([P, 3, b_c], f16, tag="qkv2")
                    push_scores((hh, Tt, vh, E9), last=True)
                    wT = load_w8("Wv", hh)
                    pps = [ps_mm.tile([P, b_c], f32, tag="mm", name=f"pv{t}") for t in range(3)]
                    mm_fp8(pps, wT, (0, 1, 2), drain_every=1)
                    for t in range(3):
                        nc.scalar.activation(vh[:, t, :], pps[t][:], AF.Copy, scale=inv)
                else:
                    vh = emit_proj("Wv", hh, 2)
                    E9 = ptt.tile([P, 3, 3, b_c], f16, tag="E9")  # [p, j, i, b]
                    push_scores((hh, Tt, vh, E9))

            # drain the remaining deferred attention items before Wo
            while work:
                drain_one()

            # ---- phase 3: output projection (fp8 DoubleRow) + residual ----
            # psum holds 512*out; residual tokens come pre-scaled by 512 and
            # the host divides the stored output by 512 (exact, power of 2).
            for ot in range(kt):
                wT = pwo.tile([P, kt, P], fp8, tag="wo")
                nc.sync.dma_start(wT[:], Wo8[ot])
                pos = [ps_mm.tile([P, b_c], f32, tag="mm", name=f"po{t}") for t in range(3)]
                for k in range(0, kt, 2):
                    for t in range(3):
                        nc.tensor.matmul(
                            pos[t][:],
                            wT[:, k : k + 2, :],
                            attT[:, k : k + 2, t, :],
                            start=(k == 0),
                            stop=(k == kt - 2),
                            perf_mode=DR,
                        )
                    if dve_work and k < kt - 4:
                        dve_work.popleft()()
                rt3 = prt3.tile([P, 3, b_c], f16, tag="rt3")
                nc.sync.dma_start(rt3[:], tokR[:, ot])
                for t in range(3):
                    oT = pout.tile([P, b_c], f32, tag="oT")
                    nc.vector.tensor_add(oT[:], pos[t][:], rt3[:, t, :])
                    nc.sync.dma_start(out[:, t, ot, :], oT[:])

    nc.compile()
    return nc


def _get_nc():
    key = "full"
    if key not in _compiled:
        _compiled[key] = _build()
    return _compiled[key]


def kernel(
    x_token,
    lat_token,
    fdbk_token,
    W_gate_L,
    b_gate_L,
    W_gate_X,
    b_gate_X,
    W_q,
    W_k,
    W_v,
    W_o,
    v_a,
):
    import ml_dtypes
    from concourse.bass_utils import run_bass_kernel_spmd

    nc = _get_nc()

    f32 = np.float32
    f16 = np.float16
    fp8 = ml_dtypes.float8_e4m3

    def wblocks(W, dtype, scale=1.0):
        # [ot, p, k, o] = W[ot*128+o, k*128+p] * scale
        a = (np.asarray(W, f32) * scale).reshape(KT, P, KT, P).transpose(0, 3, 2, 1)
        return np.ascontiguousarray(a).astype(dtype)

    w8 = {
        "WgL": wblocks(W_gate_L, fp8, WS),
        "WgX": wblocks(W_gate_X, fp8, WS),
        "Wq": wblocks(W_q, fp8, WS),
        "Wk": wblocks(W_k, fp8, WS),
        "Wv": wblocks(W_v, fp8, WS),
    }
    wo = wblocks(W_o, fp8, WS)
    bglT = np.ascontiguousarray(np.asarray(b_gate_L, f32).reshape(KT, P).T)
    bgxT = np.ascontiguousarray(np.asarray(b_gate_X, f32).reshape(KT, P).T)
    va = np.asarray(v_a, f32).reshape(H, DH)  # [h, d]
    vaR = np.zeros((DH, 2, H, P), f32)
    vaR[:, 0, :, :] = np.broadcast_to(va.T[:, :, None], (DH, H, P)) * WS
    vaR = vaR.astype(fp8)

    # tokens feature-major f16: [3, P, KT, B] then per-core batch slice
    toks = np.stack(
        [
            np.asarray(t, f32).reshape(B, KT, P).transpose(2, 1, 0)
            for t in (x_token, lat_token, fdbk_token)
        ],
        axis=0,
    ).astype(f16)  # [3, P, KT, B]

    toksR = np.ascontiguousarray(
        (toks.astype(f32) * WS).transpose(1, 2, 0, 3)
    ).astype(f16)  # pre-scaled residuals, [P, KT, 3, B]
    in_maps = []
    for c in range(N_CORES):
        s = slice(c * B_C, (c + 1) * B_C)
        m = {
            "tokf": np.ascontiguousarray(toks[:, :, :, s]),
            "tokR": np.ascontiguousarray(toksR[:, :, :, s]),
            "Wo": wo,
            "bgLT": bglT,
            "bgXT": bgxT,
            "vaR": vaR,
        }
        m.update(w8)
        in_maps.append(m)

    res = run_bass_kernel_spmd(nc, in_maps, list(range(N_CORES))).results

    # out [P, 3, KT, B_C] f32 -> [B_C, 3, D]
    full = np.concatenate(
        [res[c]["out"].transpose(3, 1, 2, 0).reshape(B_C, 3, D) for c in range(N_CORES)],
        axis=0,
    ) * np.float32(1.0 / WS)
    return tuple(np.ascontiguousarray(full[:, t : t + 1, :]) for t in range(3))
